# revision 1
# baseline (speedup 1.0000x reference)
"""Trainium2 Bass kernel for nn_MultiHeadSelfAttention_62646392979761.

Math (per the buggy-einsum reference): per position s, heads attend to heads:
  Q,K,V = x@W{q,k,v}.T + b  (N,S,H,D);  scores[s] = Q[s]K[s]^T/8 (16x16);
  A = softmax_j;  AV[s] = A[s]V[s];  out2 = scrambled reshape (16-position
  groups of one head per output row);  final = out2@Wo.T + bo.

Sharding: 8 cores x 2048 rows of the flattened (16384, 1024) x. Attention is
position-local; the scramble groups 16 consecutive positions, which never
cross a 2048-row shard. Zero cross-core communication.

Per-core device pipeline (16 tiles x 128 positions), all layouts validated
against the reference in a numpy simulator:
  1. QKV projections: PE matmuls, stationary xT e-chunks, moving fused
     [Wq/8|Wk|Wv]^T bf16; DVE bias-add evac -> QN/KN/VN bf16.
  2. xbar DMA transposes -> QT/KT chunks [(i2,d), slot].
  3. QBLK (masked block-diag pairs) + KBLK via 4 fused strided DVE copies
     each; structural zeros memset once.
  4. Scores: 64 pair-packed matmuls (k=128, m=32 col-rotated, n=16) ->
     SCO psum [(u,p,i), (j,gg)].
  5. ACT exp -> E bf16; DVE j-reduce -> Z; reciprocal -> Zr.
  6. E -> DRAM bounce -> ABLK [(p,j), (p,i,u,gg)] (masked, 32B-run DMA);
     VN -> DRAM bounce -> VTHP [(p,j), (g,d)] (128B-run DMA).
  7. AV: 64 pair matmuls (k=32, m=32 col-rotated, n=64) -> ANAT psum
     [(u,p,i), (gg,d)]; evac * Zr (per-gg tensor_scalar) -> bf16.
  8. xbar ANAT slices -> AVB2 [(b,d), (t,g',u,p,i)] accumulated all tiles.
  9. Final projection: host-permuted WoT chunks stationary, AVB2 strided
     rhs, 8-chunk psum accumulation, + bo -> outT (1024, 2048) f32.
Host: pre-permutes x rows (even/odd within tile), pre-transposes weights,
post-scatters finalT columns to (n, s_out) rows.
"""

import math
import numpy as np
import ml_dtypes

TILES = 16
ROWS = 2048
NB, SB, EB, HB, DB = 4, 4096, 1024, 16, 64

_CACHE = {}


def _split_waits_json(bir_bytes):
    """This env's walrus accepts only ONE embedded sync-wait per TPB
    instruction (NEURON_ISA_TPB_EVENTS has a single wait slot) but Tile emits
    several. Split excess on_wait entries onto standalone EventSemaphore
    instructions inserted just before, on the same engine — semantically
    identical on in-order engine queues."""
    import json
    d = json.loads(bir_bytes)
    for fn in d.get('functions', []):
        for bb in (fn.get('basic_blocks') or fn.get('blocks') or []):
            out = []
            for inst in bb.get('instructions', []):
                si = inst.get('sync_info')
                w = (si or {}).get('on_wait') or []
                if len(w) > 1:
                    for k, extra in enumerate(w[:-1]):
                        out.append({
                            'debug': inst.get('debug', 0),
                            'engine': inst['engine'],
                            'ins': [], 'outs': [],
                            'name': f"{inst['name']}-sw{k}",
                            'opcode': 'EventSemaphore',
                            'sync_info': {'on_wait': [extra], 'on_update': []},
                        })
                    si['on_wait'] = [w[-1]]
                out.append(inst)
            bb['instructions'] = out
    return json.dumps(d).encode()


def _install_birpatch():
    import concourse.bass_utils as bu
    import concourse.bass2jax as b2j
    if getattr(bu.compile_bir_kernel, '_waitsplit', False):
        return
    orig = bu.compile_bir_kernel

    def patched(bir_json, tmpdir, neff_name="file.neff"):
        return orig(_split_waits_json(bir_json), tmpdir, neff_name)

    patched._waitsplit = True
    bu.compile_bir_kernel = patched
    b2j.compile_bir_kernel = patched


def _build_bass():
    import concourse.bass as bass
    import concourse.tile as tile
    from concourse import mybir

    bf16 = mybir.dt.bfloat16
    f32 = mybir.dt.float32
    AF = mybir.ActivationFunctionType
    ALU = mybir.AluOpType
    AX = mybir.AxisListType

    nc = bass.Bass(trn_type="TRN2")
    xt_d = nc.declare_dram_parameter("xt", [1024, ROWS], bf16, isOutput=False)
    wqkv_d = nc.declare_dram_parameter("wqkv", [1024, 3072], bf16, isOutput=False)
    bias_d = nc.declare_dram_parameter("bqkv", [128, 3072], bf16, isOutput=False)
    wo_d = nc.declare_dram_parameter("wo", [1024, 1024], bf16, isOutput=False)
    bo_d = nc.declare_dram_parameter("bo", [1024, 1], f32, isOutput=False)
    out_d = nc.declare_dram_parameter("out", [1024, ROWS], f32, isOutput=True)

    from contextlib import ExitStack
    with ExitStack() as ctx:
        tc = ctx.enter_context(tile.TileContext(nc))
        const = ctx.enter_context(tc.tile_pool(name="const", bufs=1))
        work = ctx.enter_context(tc.tile_pool(name="work", bufs=2))
        psq = ctx.enter_context(tc.tile_pool(name="psq", bufs=2, space="PSUM"))
        pss = ctx.enter_context(tc.tile_pool(name="pss", bufs=1, space="PSUM"))
        drp = ctx.enter_context(tc.tile_pool(name="drp", bufs=2, space="DRAM"))

        # ---- persistent tensors ----
        wq_sb = const.tile([128, 8 * 3072], bf16, tag="wq")
        wo_sb = const.tile([128, 8 * 1024], bf16, tag="wo")
        bias_sb = const.tile([128, 3072], bf16, tag="bias")
        bo_sb = const.tile([128, 8], f32, tag="bo")
        avb2 = const.tile([128, TILES * 1024], bf16, tag="avb2")

        # weight loads: wqkv chunk ec -> cols [3072*ec : +3072]
        src = xt_d  # placeholder to appease linters
        nc.sync.dma_start(
            wq_sb[:].rearrange("p (c f) -> p c f", c=8),
            wqkv_d[:].rearrange("(c p) f -> p c f", c=8),
        )
        nc.sync.dma_start(
            wo_sb[:].rearrange("p (q f) -> p q f", q=8),
            wo_d[:].rearrange("(q p) f -> p q f", q=8),
        )
        nc.sync.dma_start(bias_sb[:], bias_d[:])
        nc.sync.dma_start(
            bo_sb[:],
            bo_d[:].rearrange("(c p) one -> p c one", c=8)[:, :, 0],
        )

        for t in range(TILES):
            # ---- load xT tile: xt_sb[:, 128c:+128] = xt_d[128c:+128, 128t:+128]
            xt_sb = work.tile([128, 1024], bf16, tag="xt")
            nc.sync.dma_start(
                xt_sb[:].rearrange("p (c s) -> p c s", c=8),
                xt_d[:].rearrange("(c p) s -> p c s", c=8)[:, :, 128 * t:128 * (t + 1)],
            )
            # ---- projections (Q, K, V sequentially through 2-bank psum) ----
            qn = work.tile([128, 1024], bf16, tag="qn")
            kn = work.tile([128, 1024], bf16, tag="kn")
            vn = work.tile([128, 1024], bf16, tag="vn")
            for w, dst in enumerate((qn, kn, vn)):
                psp = psq.tile([128, 1024], f32, tag="psp", name="psp")
                for ec in range(8):
                    lhsT = xt_sb[:, 128 * ec:128 * (ec + 1)]
                    for half in range(2):
                        rhs = wq_sb[:, 3072 * ec + 1024 * w + 512 * half:
                                    3072 * ec + 1024 * w + 512 * (half + 1)]
                        nc.tensor.matmul(
                            psp[:, 512 * half:512 * (half + 1)], lhsT, rhs,
                            start=(ec == 0), stop=(ec == 7))
                nc.vector.tensor_add(dst[:], psp[:], bias_sb[:, 1024 * w:1024 * (w + 1)])
            # ---- xbar transposes Q,K ----
            qt = work.tile([128, 1024], bf16, tag="qt")
            kt = work.tile([128, 1024], bf16, tag="kt")
            for c in range(8):
                nc.sync.dma_start_transpose(qt[:, 128 * c:128 * (c + 1)], qn[:, 128 * c:128 * (c + 1)])
                nc.sync.dma_start_transpose(kt[:, 128 * c:128 * (c + 1)], kn[:, 128 * c:128 * (c + 1)])
            # ---- QBLK / KBLK fused copies (double-buffered; re-zero masked) ----
            qblk = work.tile([128, 2048], bf16, tag="qblk", name="qblk")
            kblk = work.tile([128, 1024], bf16, tag="kblk", name="kblk")
            nc.vector.memset(qblk[:], 0.0)
            for p in range(2):
                for i2 in range(2):
                    srcq = qt[:][64 * i2:64 * i2 + 64, :].rearrange(
                        "p (c s) -> p c s", c=8)[:, :, 64 * p:64 * p + 64]
                    dstq = qblk[:][64 * p:64 * p + 64, 1024 * p:1024 * p + 1024].rearrange(
                        "p (c a g) -> p c a g", c=8, a=2)[:, :, i2, :]
                    nc.vector.tensor_copy(dstq, srcq)
                    srck = kt[:][64 * i2:64 * i2 + 64, :].rearrange(
                        "p (c s) -> p c s", c=8)[:, :, 64 * p:64 * p + 64]
                    dstk = kblk[:][64 * p:64 * p + 64, :].rearrange(
                        "p (c a g) -> p c a g", c=8, a=2)[:, :, i2, :]
                    nc.vector.tensor_copy(dstk, srck)
            # ---- scores: 64 pair matmuls ----
            sco = pss.tile([128, 256], f32, tag="sco")
            qv = qblk[:].rearrange("p (a i g) -> p a i g", a=2, i=16)
            kv = kblk[:].rearrange("p (j g) -> p j g", j=16)
            scov = sco[:].rearrange("p (j gg) -> p j gg", j=16)
            for g in range(64):
                u, gg = g % 4, g // 4
                nc.tensor.matmul(
                    scov[32 * u:32 * u + 32, :, gg],
                    qv[:, :, :, g], kv[:, :, g],
                    start=True, stop=True, tile_position=(0, 32 * u))
            # ---- softmax pieces ----
            ex = work.tile([128, 256], bf16, tag="ex")
            nc.scalar.activation(ex[:], sco[:], func=AF.Exp)
            z = work.tile([128, 16], f32, tag="z")
            zr = work.tile([128, 16], f32, tag="zr")
            nc.vector.tensor_reduce(
                z[:], ex[:].rearrange("p (j gg) -> p gg j", j=16),
                axis=AX.X, op=ALU.add)
            nc.vector.reciprocal(zr[:], z[:])
            # ---- bounce E -> ABLK ----
            ablk = work.tile([32, 2048], bf16, tag="ablk", name="ablk")
            vthp = work.tile([32, 4096], bf16, tag="vthp", name="vthp")
            nc.vector.memset(ablk[:], 0.0)
            exd = drp.tile([128, 256], bf16, tag="exd")
            nc.sync.dma_start(exd[:], ex[:])
            for p in range(2):
                for u in range(4):
                    dst = ablk[:][16 * p:16 * p + 16, 1024 * p:1024 * p + 1024].rearrange(
                        "P (i u gg) -> P i u gg", i=16, u=4)[:, :, u, :]
                    srce = exd[:].rearrange(
                        "(u a i) (j gg) -> u a j i gg", u=4, a=2, i=16, j=16)[u, p]
                    nc.sync.dma_start(dst, srce)
            # ---- bounce VN -> VTHP ----
            vnd = drp.tile([128, 1024], bf16, tag="vnd")
            nc.sync.dma_start(vnd[:], vn[:])
            for p in range(2):
                nc.sync.dma_start(
                    vthp[:][16 * p:16 * p + 16, :].rearrange("P (g d) -> P g d", g=64),
                    vnd[:].rearrange("(a g) (j d) -> a j g d", a=2, j=16)[p])
            # ---- AV: 64 pair matmuls ----
            anat = pss.tile([128, 1024], f32, tag="anat")
            av = ablk[:].rearrange("P (a i u gg) -> P a i u gg", a=2, i=16, u=4)
            vv = vthp[:].rearrange("P (g d) -> P g d", g=64)
            anv = anat[:].rearrange("p (gg d) -> p gg d", gg=16)
            for g in range(64):
                u, gg = g % 4, g // 4
                nc.tensor.matmul(
                    anv[32 * u:32 * u + 32, gg, :],
                    av[:, :, :, u, gg], vv[:, g, :],
                    start=True, stop=True, tile_position=(0, 32 * u))
            # ---- normalize by 1/Z and evac ----
            anat_sb = work.tile([128, 1024], bf16, tag="anat_sb")
            asv = anat_sb[:].rearrange("p (gg d) -> p gg d", gg=16)
            for gg in range(16):
                nc.vector.tensor_scalar_mul(asv[:, gg, :], anv[:, gg, :], zr[:, gg:gg + 1])
            # ---- xbar ANAT -> AVB2 ----
            for c4 in range(8):
                nc.sync.dma_start_transpose(
                    avb2[:, 1024 * t + 128 * c4:1024 * t + 128 * (c4 + 1)],
                    anat_sb[:, 128 * c4:128 * (c4 + 1)])

        # ---- final projection ----
        avv = avb2[:].rearrange("p (t c u a i) -> p t c u a i", t=TILES, c=8, u=4, a=2)
        for fc in range(8):
            for tg in range(4):
                psf = pss.tile([128, 512], f32, tag="psf")
                for q in range(8):
                    u, pq = q // 2, q % 2
                    nc.tensor.matmul(
                        psf[:], wo_sb[:, 1024 * q + 128 * fc:1024 * q + 128 * (fc + 1)],
                        avv[:, 4 * tg:4 * (tg + 1), :, u, pq, :],
                        start=(q == 0), stop=(q == 7))
                osb = work.tile([128, 512], f32, tag="osb")
                nc.vector.tensor_scalar_add(osb[:], psf[:], bo_sb[:, fc:fc + 1])
                nc.sync.dma_start(out_d[128 * fc:128 * (fc + 1), 512 * tg:512 * (tg + 1)], osb[:])
    return nc


def _host_prep(x, Wq, bq, Wk, bk, Wv, bv, Wo, bo):
    """Returns per-core input maps + post-scatter metadata."""
    xf = np.ascontiguousarray(x.reshape(NB * SB, EB))
    # slot permutation within each 128-tile: col 128t + 64p + g <- row 128t+2g+p
    idx = np.arange(ROWS)
    t, r = idx // 128, idx % 128
    p, g = r // 64, r % 64
    perm = 128 * t + 2 * g + p
    Wqs, bqs = Wq / 8.0, bq / 8.0
    WQKVT = np.concatenate([Wqs.T, Wk.T, Wv.T], axis=1).astype(ml_dtypes.bfloat16)
    BQKV = np.tile(np.concatenate([bqs, bk, bv])[None, :], (128, 1)).astype(ml_dtypes.bfloat16)
    WoTP = np.zeros((1024, 1024), np.float32)
    for u in range(4):
        for pp in range(2):
            q = 2 * u + pp
            for b in range(2):
                m = 8 * b + 2 * u + pp
                WoTP[q * 128 + b * 64:q * 128 + (b + 1) * 64, :] = Wo[:, m * 64:(m + 1) * 64].T
    WoTP = WoTP.astype(ml_dtypes.bfloat16)
    boT = bo.reshape(1024, 1).astype(np.float32)
    in_maps = []
    for core in range(8):
        n, s0 = core // 2, (core % 2) * ROWS
        xs = xf[n * SB + s0:n * SB + s0 + ROWS][perm]
        xT = np.ascontiguousarray(xs.T).astype(ml_dtypes.bfloat16)
        in_maps.append({"xt": xT, "wqkv": WQKVT, "bqkv": BQKV, "wo": WoTP, "bo": boT})
    return in_maps


def kernel(x, Wq, bq, Wk, bk, Wv, bv, Wo, bo):
    _install_birpatch()
    from concourse.bass_utils import run_bass_kernel_spmd

    if "nc" not in _CACHE:
        _CACHE["nc"] = _build_bass()
    nc = _CACHE["nc"]
    in_maps = _host_prep(np.asarray(x, np.float32), *[np.asarray(a, np.float32)
                         for a in (Wq, bq, Wk, bk, Wv, bv, Wo, bo)])
    res = run_bass_kernel_spmd(nc, in_maps, list(range(8)))
    out = np.zeros((NB, SB, EB), np.float32)
    # col t*128 + c4*16 + h -> row s_out = h*256 + (s0/16 + 8t + c4)
    tt = np.arange(ROWS)
    ct, cc4, ch = tt // 128, (tt // 16) % 8, tt % 16
    for core in range(8):
        n, s0 = core // 2, (core % 2) * ROWS
        fT = np.asarray(res.results[core]["out"])  # (1024, 2048)
        rows = ch * 256 + (s0 // 16 + 8 * ct + cc4)
        out[n, rows, :] = fT.T
    return out



# revision 5
# speedup vs baseline: 225.9094x; 225.9094x over previous
"""Trainium2 Bass kernel v2 for nn_MultiHeadSelfAttention_62646392979761.

Math (buggy-einsum reference): per position s, heads attend to heads:
  Q,K,V = x@W.T + b (N,S,H,D); scores[s] = Q[s]K[s]^T/8 (16x16);
  A = softmax_h2; av[s] = A[s]V[s]; out2 = scrambled reshape; final = out2@Wo.T+bo.

Sharding: 8 cores x 2048 positions of flattened (16384, 1024) x. Zero
cross-core communication.

Per-core pipeline (s-blocks of 512 = 4 tiles of 128 positions):
  1. Q/K projections W-stationary -> QT/KT [f-chunk=(hh,d), (c,s)] directly
     (no transposes). V projection xT-stationary -> VN [s, f] natural.
  2. VN -> DRAM -> VSTK [(u,a,h2), (gg,d)] shuffle (1 write + 4 reads).
  3. QSTK/KBLK pair blocks via 4+4 strided DVE copies from QT/KT.
  4. Scores TRANSPOSED: 64 pair-MMs -> SCO' [(u,a,h2), (gg,h1)] psum.
  5. exp on ACT -> E'; Z via ones-matmul; reciprocal; broadcast 1/Z across
     h2 partitions via second ones-matmul; A' = E' * ZB (one DVE mul).
  6. ABLK block-diag (8 small DVE copies, partition-aligned with A').
  7. AV: 64 MMs, V-stationary, out = av^T at tile_position (32u, 64beta)
     -> AVT [(beta,d), (gam,u,a,h1)] psum -> avt_sb (no transpose needed).
  8. O-projection: W2 chunks stationary (j-pairs stacked on (beta,d)),
     AVT slices moving (N=256), 16 accumulating MMs -> +bo -> out store.
Position encoding per tile: local row r = 2g+a, pair g = 16u+gg.
Output column = 16*t_global + h1; host scatters rows 256*h1 + t0 + t_global.
"""

import math
import numpy as np
import ml_dtypes

TILES = 16
ROWS = 2048
NB, SB, EB, HB, DB = 4, 4096, 1024, 16, 64
REPS = 1  # benchmark knob: repeat the compute body REPS times

_CACHE = {}


def _split_waits_json(bir_bytes):
    """Walrus here accepts only ONE embedded sync-wait per TPB instruction but
    Tile emits several; split extras onto standalone EventSemaphore insts."""
    import json
    d = json.loads(bir_bytes)
    for fn in d.get('functions', []):
        for bb in (fn.get('basic_blocks') or fn.get('blocks') or []):
            out = []
            for inst in bb.get('instructions', []):
                si = inst.get('sync_info')
                w = (si or {}).get('on_wait') or []
                if len(w) > 1:
                    for k, extra in enumerate(w[:-1]):
                        out.append({
                            'debug': inst.get('debug', 0),
                            'engine': inst['engine'],
                            'ins': [], 'outs': [],
                            'name': f"{inst['name']}-sw{k}",
                            'opcode': 'EventSemaphore',
                            'sync_info': {'on_wait': [extra], 'on_update': []},
                        })
                    si['on_wait'] = [w[-1]]
                out.append(inst)
            bb['instructions'] = out
    return json.dumps(d).encode()


def _install_birpatch():
    import concourse.bass_utils as bu
    import concourse.bass2jax as b2j
    if getattr(bu.compile_bir_kernel, '_waitsplit', False):
        return
    orig = bu.compile_bir_kernel

    def patched(bir_json, tmpdir, neff_name="file.neff"):
        return orig(_split_waits_json(bir_json), tmpdir, neff_name)

    patched._waitsplit = True
    bu.compile_bir_kernel = patched
    b2j.compile_bir_kernel = patched


def _build_bass(debug=False):
    import concourse.bass as bass
    import concourse.tile as tile
    from concourse import mybir

    bf16 = mybir.dt.bfloat16
    f32 = mybir.dt.float32
    AF = mybir.ActivationFunctionType

    nc = bass.Bass(trn_type="TRN2")
    xt_d = nc.declare_dram_parameter("xt", [1024, ROWS], bf16, isOutput=False)
    wqk_d = nc.declare_dram_parameter("wqk", [1024, 2048], bf16, isOutput=False)
    wv_d = nc.declare_dram_parameter("wv", [1024, 1024], bf16, isOutput=False)
    w2_d = nc.declare_dram_parameter("w2", [128, 8192], bf16, isOutput=False)
    bqk_d = nc.declare_dram_parameter("bqk", [128, 18], f32, isOutput=False)
    bv_d = nc.declare_dram_parameter("bv", [128, 1024], bf16, isOutput=False)
    bo_d = nc.declare_dram_parameter("bo", [128, 8], f32, isOutput=False)
    ones_d = nc.declare_dram_parameter("ones", [128, 136], bf16, isOutput=False)
    out_d = nc.declare_dram_parameter("out", [1024, ROWS], f32, isOutput=True)
    dbg = {}
    if debug:
        for nm, shape, dt in [("dqt", [128, 4096], bf16), ("dkt", [128, 4096], bf16),
                              ("dvn", [128, 1024], bf16), ("dvstk", [128, 1024], bf16),
                              ("dsco", [128, 256], f32), ("dasb", [128, 256], bf16),
                              ("davt", [128, 1024], bf16)]:
            dbg[nm] = nc.declare_dram_parameter(nm, shape, dt, isOutput=True)

    from contextlib import ExitStack
    with ExitStack() as ctx:
        tc = ctx.enter_context(tile.TileContext(nc))
        const = ctx.enter_context(tc.tile_pool(name="const", bufs=1))
        blk = ctx.enter_context(tc.tile_pool(name="blk", bufs=2))
        work = ctx.enter_context(tc.tile_pool(name="work", bufs=2))
        psP = ctx.enter_context(tc.tile_pool(name="psP", bufs=2, space="PSUM"))
        psS = ctx.enter_context(tc.tile_pool(name="psS", bufs=1, space="PSUM"))
        psV = ctx.enter_context(tc.tile_pool(name="psV", bufs=1, space="PSUM"))
        psO = ctx.enter_context(tc.tile_pool(name="psO", bufs=1, space="PSUM"))
        drp = ctx.enter_context(tc.tile_pool(name="drp", bufs=2, space="DRAM"))

        # ---- persistent tensors ----
        wq_ab = [const.tile([128, 4096], bf16, tag=f"wqab{i}", name=f"wqab{i}")
                 for i in range(2)]   # [p=e%128, (e-half, f)]
        wk_sb = const.tile([128, 8 * 1024], bf16, tag="wk")
        wv_sb = const.tile([128, 8 * 1024], bf16, tag="wv")         # [p=e%128,(ec,f)]
        w2_sb = const.tile([128, 8192], bf16, tag="w2")             # [p=(beta,d),(gm,a,f)]
        bqk_sb = const.tile([128, 18], f32, tag="bqk")              # [p=f%128,(w,c)]
        bv_sb = const.tile([128, 1024], bf16, tag="bv")
        bo_sb = const.tile([128, 8], f32, tag="bo")
        ones_sb = const.tile([128, 136], bf16, tag="ones")
        avt_sb = const.tile([128, TILES * 1024], bf16, tag="avt")
        kblk_bufs = [const.tile([128, 2048], bf16, tag=f"kblk{i}", name=f"kblk{i}")
                     for i in range(2)]
        for kb in kblk_bufs:
            nc.vector.memset(kb[:], 0.0)

        for i in range(2):
            nc.sync.dma_start(
                wq_ab[i][:].rearrange("p (c f) -> p c f", c=4),
                wqk_d[:].rearrange("(c p) (k f) -> p k c f", c=8, k=2)[
                    :, 0, 4 * i:4 * (i + 1)])
        nc.sync.dma_start(bqk_sb[:], bqk_d[:])
        h_wk = nc.scalar.dma_start(
            wk_sb[:].rearrange("p (c f) -> p c f", c=8),
            wqk_d[:].rearrange("(c p) (k f) -> p k c f", c=8, k=2)[:, 1])
        h_wv = nc.gpsimd.dma_start(
            wv_sb[:].rearrange("p (c f) -> p c f", c=8),
            wv_d[:].rearrange("(c p) f -> p c f", c=8))
        nc.gpsimd.dma_start(bv_sb[:], bv_d[:])
        nc.gpsimd.dma_start(ones_sb[:], ones_d[:])
        h_w2 = nc.gpsimd.dma_start(w2_sb[:], w2_d[:])
        nc.gpsimd.dma_start(bo_sb[:], bo_d[:])
        from concourse.tile import add_dep_helper
        # stage big weight loads behind the critical first wave (wq+xtb):
        # SDMA round-robins packets fairly, so anything enqueued early delays
        # the first matmul's inputs
        add_dep_helper(h_wv.ins, h_wk.ins, sync=False,
                       reason="stage wv behind wk")
        add_dep_helper(h_w2.ins, h_wv.ins, sync=False,
                       reason="stage w2 behind wv")

        for rep in range(REPS):
          for b4 in range(4):  # s-block of 512 positions = 4 tiles
            # ---- load xT block [p=e%128, (ec, s512)] ----
            xtb_ab = [blk.tile([128, 2048], bf16, tag=f"xtab{i}", name=f"xtab{i}")
                      for i in range(2)]
            for i in range(2):
                h_x = nc.sync.dma_start(
                    xtb_ab[i][:].rearrange("p (c s) -> p c s", c=4),
                    xt_d[:].rearrange("(c p) s -> p c s", c=8)[
                        :, 4 * i:4 * (i + 1), 512 * b4:512 * (b4 + 1)])
                if b4 == 0 and i == 1:
                    add_dep_helper(h_wk.ins, h_x.ins, sync=False,
                                   reason="stage wk behind first x wave")
            # ---- Q/K projections (W-stationary) -> QTb/KTb [p=(hh,d),(c,s)] ----
            qtb = blk.tile([128, 8 * 512], bf16, tag="qtb")
            ktb = blk.tile([128, 8 * 512], bf16, tag="ktb")
            for w, dst in ((0, qtb), (1, ktb)):
                for c in range(8):
                    pq = psP.tile([128, 512], f32, tag="pp", name="pq")
                    for e in range(8):
                        i, el = e // 4, e % 4
                        wsl = (wq_ab[i][:, 1024 * el + 128 * c:1024 * el + 128 * (c + 1)]
                               if w == 0 else
                               wk_sb[:, 1024 * e + 128 * c:1024 * e + 128 * (c + 1)])
                        nc.tensor.matmul(
                            pq[:], wsl,
                            xtb_ab[i][:, 512 * el:512 * (el + 1)],
                            start=(e == 0), stop=(e == 7))
                    nc.scalar.activation(
                        dst[:, 512 * c:512 * (c + 1)], pq[:],
                        func=AF.Identity, bias=bqk_sb[:, 8 * w + c:8 * w + c + 1])
            if debug and b4 == 0:
                nc.sync.dma_start(dbg["dqt"][:], qtb[:])
                nc.sync.dma_start(dbg["dkt"][:], ktb[:])
            # ---- V projection (xT-stationary) + middle, per tile ----
            for tt in range(4):
                t = 4 * b4 + tt
                vn = work.tile([128, 1024], bf16, tag="vn", name="vn")
                for hf in range(2):
                    pvh = psP.tile([128, 512], f32, tag="pp", name="pvh")
                    for e in range(8):
                        i, el = e // 4, e % 4
                        nc.tensor.matmul(
                            pvh[:],
                            xtb_ab[i][:, 512 * el + 128 * tt:512 * el + 128 * (tt + 1)],
                            wv_sb[:, 1024 * e + 512 * hf:1024 * e + 512 * (hf + 1)],
                            start=(e == 0), stop=(e == 7))
                    nc.vector.tensor_add(
                        vn[:, 512 * hf:512 * (hf + 1)], pvh[:],
                        bv_sb[:, 512 * hf:512 * (hf + 1)])
                if debug and t == 0:
                    nc.sync.dma_start(dbg["dvn"][:], vn[:])
                # ---- VSTK via DRAM bounce ----
                vd = drp.tile([128, 1024], bf16, tag="vd")
                nc.scalar.dma_start(vd[:], vn[:])
                vstk = work.tile([128, 1024], bf16, tag="vstk", name="vstk")
                # vd rows r=32u+2gg+a, cols 64h2+d ; vstk[p=32u+16a+h2,(gg,d)]
                vsrc = vd[:].rearrange("(u g a) (h d) -> u (a h) g d",
                                       u=4, g=16, a=2, h=16)
                for u in range(4):
                    eng = nc.sync if u < 2 else nc.gpsimd
                    eng.dma_start(
                        vstk[:][32 * u:32 * (u + 1), :].rearrange(
                            "p (g d) -> p g d", g=16),
                        vsrc[u])
                if debug and t == 0:
                    nc.sync.dma_start(dbg["dvstk"][:], vstk[:])
                # ---- QSTK / KBLK pair blocks ----
                qstk = work.tile([128, 1024], bf16, tag="qstk", name="qstk")
                kblk = kblk_bufs[t % 2]
                for a in range(2):
                    for hh in range(2):
                        qsrc = qtb[:][64 * hh:64 * (hh + 1), :].rearrange(
                            "p (c t g a) -> p g c t a", c=8, t=4, g=64)[:, :, :, tt, a]
                        qdst = qstk[:][64 * a:64 * (a + 1), :].rearrange(
                            "p (g two c) -> p g two c", g=64, two=2, c=8)[:, :, hh, :]
                        nc.vector.tensor_copy(qdst, qsrc)
                        ksrc = ktb[:][64 * hh:64 * (hh + 1), :].rearrange(
                            "p (c t g a) -> p g c t a", c=8, t=4, g=64)[:, :, :, tt, a]
                        kdst = kblk[:][64 * a:64 * (a + 1), :].rearrange(
                            "p (g a2 two c) -> p g a2 two c",
                            g=64, a2=2, two=2, c=8)[:, :, a, hh, :]
                        nc.vector.tensor_copy(kdst, ksrc)
                # ---- scores (transposed): 64 pair MMs -> SCO' psum ----
                sco = psS.tile([128, 256], f32, tag="sco")
                for g in range(64):
                    u, gg = g // 16, g % 16
                    nc.tensor.matmul(
                        sco[32 * u:32 * (u + 1), 16 * gg:16 * (gg + 1)],
                        kblk[:, 32 * g:32 * (g + 1)],
                        qstk[:, 16 * g:16 * (g + 1)],
                        start=True, stop=True, tile_position=(0, 32 * u))
                if debug and t == 0:
                    scodbg = work.tile([128, 256], f32, tag="scodbg")
                    nc.vector.tensor_copy(scodbg[:], sco[:])
                    nc.sync.dma_start(dbg["dsco"][:], scodbg[:])
                # ---- softmax: exp, Z (ones-MM), 1/Z, broadcast-MM, A' ----
                esb = work.tile([128, 256], bf16, tag="esb", name="esb")
                nc.scalar.activation(esb[:], sco[:], func=AF.Exp)
                zps = psS.tile([8, 256], f32, tag="zz", name="zps")
                nc.tensor.matmul(zps[:], ones_sb[:, 0:8], esb[:],
                                 start=True, stop=True)
                zr = work.tile([8, 256], bf16, tag="zr", name="zr")
                with nc.allow_low_precision(reason="1/Z in bf16 is ample for softmax"):
                    nc.vector.reciprocal(zr[:], zps[:])
                zb = psS.tile([128, 256], f32, tag="zz", name="zb")
                nc.tensor.matmul(zb[:], ones_sb[:][0:8, 8:136], zr[:],
                                 start=True, stop=True)
                asb = work.tile([128, 256], bf16, tag="asb", name="asb")
                nc.vector.tensor_mul(asb[:], esb[:], zb[:])
                if debug and t == 0:
                    nc.sync.dma_start(dbg["dasb"][:], asb[:])
                # ---- ABLK block-diag [p=(u,a,h2), (gg, a', h1)] ----
                # full-partition copy per a' with a per-partition 0/1 mask
                # (mask[p, a'] = 1 iff partition's a == a') zeroing off-diag
                ablk = work.tile([128, 512], bf16, tag="ablk", name="ablk")
                for a2 in range(2):
                    adst = ablk[:].rearrange(
                        "p (g a2 h) -> p g a2 h", g=16, a2=2)[:, :, a2, :]
                    nc.vector.tensor_scalar_mul(
                        adst, asb[:], bqk_sb[:, 16 + a2:17 + a2])
                # ---- AV: 64 MMs, V-stationary -> AVT psum ----
                avt = psV.tile([128, 1024], f32, tag="avt", name="avtps")
                for u in range(4):
                    for gg in range(16):
                        gam, beta = gg // 2, gg % 2
                        nc.tensor.matmul(
                            avt[64 * beta:64 * (beta + 1),
                                128 * gam + 32 * u:128 * gam + 32 * (u + 1)],
                            vstk[32 * u:32 * (u + 1), 64 * gg:64 * (gg + 1)],
                            ablk[32 * u:32 * (u + 1), 32 * gg:32 * (gg + 1)],
                            start=True, stop=True, tile_position=(32 * u, 64 * beta))
                nc.vector.tensor_copy(
                    avt_sb[:].rearrange("p (g t r) -> p g t r", g=8, t=TILES)[:, :, t, :],
                    avt[:].rearrange("p (g r) -> p g r", g=8))
                if debug and t == 0:
                    nc.sync.dma_start(
                        dbg["davt"][:].rearrange("p (g r) -> p g r", g=8),
                        avt_sb[:].rearrange("p (g t r) -> p g t r", g=8, t=TILES)[:, :, 0, :])
            # ---- O-projection: half-core N=512 after b4=1, quarters after ----
            ojobs = []
            if b4 == 1:
                ojobs = [(0, 8, 0)]       # (tile0, ntiles, colbase)
            elif b4 >= 2:
                ojobs = [(4 * b4, 4, 512 * b4)]
            for (t0, nt, cb) in ojobs:
              for fc in range(8):
                pos = [psO.tile([128, 512], f32, tag=f"po{g4}", name=f"po{g4}")
                       for g4 in range(2)]
                for gm in range(4):
                    for a in range(2):
                        wsl = w2_sb[:, 2048 * gm + 1024 * a + 128 * fc:
                                    2048 * gm + 1024 * a + 128 * (fc + 1)]
                        for g4 in range(2):
                            gam = gm + 4 * g4
                            rhs = avt_sb[:].rearrange(
                                "p (gm8 t u a2 h) -> p gm8 t u a2 h",
                                gm8=8, t=TILES, u=4, a2=2)[
                                :, gam, t0:t0 + nt, :, a, :]
                            nc.tensor.matmul(pos[g4][:, 0:64 * nt], wsl, rhs,
                                             start=(gm == 0 and a == 0),
                                             stop=(gm == 3 and a == 1))
                osb = work.tile([128, 1024], f32, tag="osb")
                for g4 in range(2):
                    nc.vector.tensor_scalar_add(
                        osb[:, 64 * nt * g4:64 * nt * (g4 + 1)],
                        pos[g4][:, 0:64 * nt], bo_sb[:, fc:fc + 1])
                nc.sync.dma_start(
                    out_d[128 * fc:128 * (fc + 1), cb:cb + 128 * nt],
                    osb[:, 0:128 * nt])
    return nc


def _host_prep(x, Wq, bq, Wk, bk, Wv, bv, Wo, bo):
    bf = ml_dtypes.bfloat16
    xf = np.ascontiguousarray(x.reshape(NB * SB, EB))
    Wqs, bqs = Wq / 8.0, bq / 8.0
    WQKT = np.concatenate([Wqs.T, Wk.T], axis=1).astype(bf)        # [1024, 2048]
    # V columns in h2' = 8*hh + c slot order (head 2c+hh at slot 8hh+c),
    # matching the (hh, c) head enumeration of the K-chunk layout
    slot = np.arange(16)
    headof = 2 * (slot % 8) + slot // 8
    vperm = (headof[:, None] * 64 + np.arange(64)[None, :]).reshape(-1)
    WVT = np.ascontiguousarray(Wv.T[:, vperm]).astype(bf)          # [1024, 1024]
    BQK = np.zeros((128, 18), np.float32)
    p = np.arange(128)
    BQK[:, 16] = ((p // 16) % 2 == 0)
    BQK[:, 17] = ((p // 16) % 2 == 1)
    for c in range(8):
        BQK[:, c] = bqs[128 * c:128 * (c + 1)]
        BQK[:, 8 + c] = bk[128 * c:128 * (c + 1)]
    BV = np.tile(bv[vperm][None, :], (128, 1)).astype(bf)
    # w2: [p=(beta,d), (gm, a, f)] = Wo[f, 64*j+d], j = 4gm+2beta+a
    W2 = np.zeros((128, 8192), np.float32)
    for gm in range(4):
        for a in range(2):
            for beta in range(2):
                j = 4 * gm + 2 * beta + a
                W2[64 * beta:64 * (beta + 1),
                   1024 * (2 * gm + a):1024 * (2 * gm + a + 1)] = \
                    Wo[:, 64 * j:64 * (j + 1)].T
    W2 = W2.astype(bf)
    BO = np.zeros((128, 8), np.float32)
    for c in range(8):
        BO[:, c] = bo[128 * c:128 * (c + 1)]
    # ones: cols 0-8: ONES8[p=(u,a,h2), m=(u',a')] = delta((u,a),(u',a'))
    #       rows 0-8, cols 8-136: ONESB[p=(u,a), m=(u',a',h2)] = delta
    ONES = np.zeros((128, 136), np.float32)
    for u in range(4):
        for a in range(2):
            q8 = 2 * u + a
            ONES[32 * u + 16 * a:32 * u + 16 * (a + 1), q8] = 1.0
            ONES[q8, 8 + 32 * u + 16 * a:8 + 32 * u + 16 * (a + 1)] = 1.0
    ONES = ONES.astype(bf)
    in_maps = []
    for core in range(8):
        n, s0 = core // 2, (core % 2) * ROWS
        xs = xf[n * SB + s0:n * SB + s0 + ROWS]
        xT = np.ascontiguousarray(xs.T).astype(bf)
        in_maps.append({"xt": xT, "wqk": WQKT, "wv": WVT, "w2": W2,
                        "bqk": BQK, "bv": BV, "bo": BO, "ones": ONES})
    return in_maps


def _scatter_rows(core):
    """Output column c of core -> (n, row) in the full output.
    c = 512*b4 + 256*g4 + 64*tt + 16*u + h1 ; t_global = 32*b4+8*tt+2*u+g4."""
    n, s0 = core // 2, (core % 2) * ROWS
    c = np.arange(ROWS)
    u, hs = (c // 16) % 4, c % 16
    h1 = 2 * (hs % 8) + hs // 8   # head at column slot hs = 8*hh + c
    # cols 0-1024: half-core job (nt=8); cols >= 1024: quarter jobs (nt=4)
    lo = c < 1024
    g4 = np.where(lo, (c // 512) % 2, (c % 512) // 256)
    tt = np.where(lo, (c // 64) % 8, 4 * (c // 512) + (c // 64) % 4)
    tg = 8 * tt + 2 * u + g4
    rows = 256 * h1 + (s0 // 16) + tg
    return n, rows


def kernel(x, Wq, bq, Wk, bk, Wv, bv, Wo, bo):
    _install_birpatch()
    from concourse.bass_utils import run_bass_kernel_spmd

    if "nc" not in _CACHE:
        _CACHE["nc"] = _build_bass()
    nc = _CACHE["nc"]
    in_maps = _host_prep(np.asarray(x, np.float32), *[np.asarray(a, np.float32)
                         for a in (Wq, bq, Wk, bk, Wv, bv, Wo, bo)])
    res = run_bass_kernel_spmd(nc, in_maps, list(range(8)))
    out = np.zeros((NB, SB, EB), np.float32)
    for core in range(8):
        n, rows = _scatter_rows(core)
        fT = np.asarray(res.results[core]["out"])  # (1024, 2048)
        out[n, rows, :] = fT.T
    return out
